# revision 48
# baseline (speedup 1.0000x reference)
"""Trainium2 Bass kernel for causal multi-head attention (B=2, S=2048, D=1024, 16 heads x 64).

Sharding: 8 cores = 2 batches x 4 head-groups (tensor parallel over heads).
Each core computes attention for its 4 heads over the full sequence; the 4
cores of a batch AllGather normalized head outputs per q-quarter (4-rank
groups), and each core applies the full W_O projection to one 128-row block
of every quarter (so only the final AllGather gates a small slice of work).
Host concatenates the interleaved blocks.

Attention is flash-style with transposed scores: sT[k, q] = K Q^T (keys on
partitions). Both heads of a pair write one [128, 1024] 2-bank PSUM tile so a
single ACT exp covers them. AV uses stationary [v | 1] so PSUM row 64
accumulates the softmax denominator for free; the denominator row is
broadcast to 64 partitions by a 1-contraction fp32r matmul and reciprocated
at base partition 0 with the fast custom DVE op (nonzero-base custom DVE
silently no-ops). Head-pairs (pr) run sequentially per q-tile so attention
needs only 2 accumulator banks. QKV projection and out-projection matmul
chains are chopped into single-instruction units and interleaved between
attention rounds so the PE FIFO always has dependency-free filler during
ACT-bound stretches (keeps HAM warm, overlaps phases).
"""

import os
import sys

sys.path.insert(0, "/opt/trn_rl_repo")

import numpy as np

# ---- problem constants (hardcoded; kernel.py must be self-contained) ----
B = 2
S = 2048
D = 1024
N_HEADS = 16
DH = 64                 # head dim
NCORES = 8
NH_CORE = N_HEADS // 4  # 4 heads per core (4-way TP x 2-way batch DP)
SCALE = 1.0 / 8.0       # 1/sqrt(64)

P = 128                 # partitions
DC = D // P             # 8 contraction chunks for the projections
KC = S // P             # 16 key chunks
QT = 512                # q tile width (free dim) per quarter
NQT = S // QT           # 4 q tiles
GRP = 4                 # cores per batch group

_CACHE = {}


def _build():
    import concourse.bass as bass
    import concourse.tile as tile
    from concourse import bacc, mybir

    f32 = mybir.dt.float32
    f32r = mybir.dt.float32r
    F16 = mybir.dt.float16

    nc = bacc.Bacc(
        "TRN2",
        target_bir_lowering=False,
        debug=False,
        enable_asserts=False,
        num_devices=NCORES,
    )

    # all inputs pre-chunked host-side to [128, DC, n] so DMAs are contiguous
    xt_d = nc.dram_tensor("xt", [P, DC, S], F16, kind="ExternalInput").ap()
    wqt_d = nc.dram_tensor("wqt", [P, DC, NH_CORE * DH], F16, kind="ExternalInput").ap()
    wkt_d = nc.dram_tensor("wkt", [P, DC, NH_CORE * DH], F16, kind="ExternalInput").ap()
    wvt_d = nc.dram_tensor("wvt", [P, DC, NH_CORE * DH], F16, kind="ExternalInput").ap()
    wof_d = nc.dram_tensor("wof", [P, DC, D], F16, kind="ExternalInput").ap()
    msk_d = nc.dram_tensor("msk", [P, P], F16, kind="ExternalInput").ap()
    # 4 blocks of 128 rows: block q = rows [q*512 + g*128 .. +128] of this
    # core's batch output (g = group rank)
    out_d = nc.dram_tensor("out", [NQT * P, D], F16, kind="ExternalOutput").ap()
    exp_pair = int(os.environ.get("KERNEL_EXPPAIR", "1"))

    Exp = mybir.ActivationFunctionType.Exp

    with tile.TileContext(nc) as tc:
        with (
            tc.tile_pool(name="const", bufs=1) as const,
            tc.tile_pool(name="work", bufs=2) as work,
            tc.tile_pool(name="ps", bufs=1, space="PSUM") as ps_pool,
            tc.tile_pool(name="dram", bufs=1, space="DRAM") as dram,
        ):
            # ---------------- input DMAs ----------------
            wq_sb = const.tile([P, DC, NH_CORE * DH], F16)
            nc.sync.dma_start(wq_sb[:], wqt_d)
            wk_sb = const.tile([P, DC, NH_CORE * DH], F16)
            nc.sync.dma_start(wk_sb[:], wkt_d)

            # residual^T: first quarter per d-chunk (small, on gpsimd queue so
            # issue overlaps the sync-engine weight DMAs), rest in fat slices
            # issue the first-quarter chunks from two idle engine queues in
            # parallel (each dma_start costs ~0.6us of issue time on its
            # engine; serializing all 8 on one engine delays the first matmul)
            xt_sb = const.tile([P, DC, S], F16)
            for dc in range(DC):
                eng = nc.gpsimd if dc % 2 == 0 else nc.scalar
                eng.dma_start(xt_sb[:, dc, 0:QT], xt_d[:, dc, 0:QT])
            tri_sb = const.tile([P, P], F16)
            nc.sync.dma_start(tri_sb[:], msk_d)
            wv_sb = const.tile([P, DC, NH_CORE * DH], F16)
            nc.sync.dma_start(wv_sb[:], wvt_d)
            # pair chunks into 4 wide transfers split across two queues so the
            # quarter-1 filler chains aren't gated on serial DMA issue
            for dc in range(0, DC, 2):
                eng = nc.gpsimd if dc % 4 == 0 else nc.scalar
                eng.dma_start(
                    xt_sb[:, dc : dc + 2, QT:S], xt_d[:, dc : dc + 2, QT:S]
                )

            # ---------------- SBUF state ----------------
            qT = [const.tile([P, S], F16, name=f"qT{i}") for i in range(2)]
            kT = [const.tile([P, S], F16, name=f"kT{i}") for i in range(2)]
            v_aug = [const.tile([P, KC, DH + 1], F16, name=f"vaug{h}") for h in range(NH_CORE)]
            ones_f32 = const.tile([P, DH], f32)
            nc.vector.memset(ones_f32[:], 1.0)
            ones_f16 = const.tile([DH + 1, DH], F16)
            nc.vector.memset(ones_f16[:], 1.0)
            for h in range(NH_CORE):
                nc.scalar.copy(v_aug[h][:, :, DH : DH + 1], ones_f32[:, 0:KC, None])

            wo_sb = const.tile([P, DC, D], F16)

            cc_in = dram.tile([NQT * 2 * P, QT], F16)
            cc_out = dram.tile([NQT * GRP * 2 * P, QT], F16)

            pj = [0]  # alternating tag counter for the 2 shared psum banks

            def _pj_tile(shape, name):
                t = ps_pool.tile(shape, f32, name=name, tag=f"pj{pj[0] % 2}", bufs=1)
                pj[0] += 1
                return t

            # ---- filler units: single instructions emitted between rounds ----
            def qk_chain_units(nt, pr, w_sb, dst):
                st = {}
                us = []
                for dc in range(DC):
                    def mm(dc=dc, nt=nt, pr=pr, w_sb=w_sb):
                        if dc == 0:
                            st["pp"] = _pj_tile([P, QT], "pp")
                        nc.tensor.matmul(
                            st["pp"][:],
                            w_sb[:, dc, pr * P : (pr + 1) * P],
                            xt_sb[:, dc, nt * QT : (nt + 1) * QT],
                            start=(dc == 0),
                            stop=(dc == DC - 1),
                        )
                    us.append(mm)
                def cp(nt=nt, dst=dst):
                    nc.vector.tensor_copy(dst[:, nt * QT : (nt + 1) * QT], st["pp"][:])
                us.append(cp)
                return us

            def v_chain_units(pc):
                st = {}
                us = []
                for dc in range(DC):
                    def mm(dc=dc, pc=pc):
                        if dc == 0:
                            st["vp"] = _pj_tile([P, NH_CORE * DH], "vp")
                        nc.tensor.matmul(
                            st["vp"][:],
                            xt_sb[:, dc, pc * P : (pc + 1) * P],
                            wv_sb[:, dc, :],
                            start=(dc == 0),
                            stop=(dc == DC - 1),
                        )
                    us.append(mm)
                for h in range(NH_CORE):
                    def cp(h=h, pc=pc):
                        nc.vector.tensor_copy(
                            v_aug[h][:, pc, 0:DH], st["vp"][:, h * DH : (h + 1) * DH]
                        )
                    us.append(cp)
                return us

            def proj_units(nt, prs=(0, 1), with_v=True):
                us = []
                for pr in prs:
                    us += qk_chain_units(nt, pr, wq_sb, qT[pr])
                    us += qk_chain_units(nt, pr, wk_sb, kT[pr])
                if with_v:
                    for pc in range(4 * nt, 4 * nt + 4):
                        us += v_chain_units(pc)
                return us

            # my 128-q-column block within each quarter (dynamic by rank)
            pid = nc.partition_id()
            qoff = nc.snap(
                nc.s_assert_within(
                    (pid % GRP) * P, 0, QT - P, skip_runtime_assert=True
                )
            )
            cc_q = cc_out[:].rearrange("(t a p) q -> t p a q", p=P, a=GRP * 2)

            def out_proj_units(qt):
                """Two phases: (a) after gather(qt, pr=0) — attR chunks 0-3
                (W_O chunks 0,2,4,6); (b) after gather(qt, pr=1) — chunks 4-7
                (W_O chunks 1,3,5,7) + evacuate + store. PSUM accumulates
                across the phase boundary."""
                st = {}
                ua, ub = [], []
                def dma_a(qt=qt):
                    st["attR"] = work.tile([P, GRP * 2, P], F16, name="attR", bufs=2)
                    nc.sync.dma_start(
                        st["attR"][:, 0:GRP, :], cc_q[qt, :, 0:GRP, bass.ds(qoff, P)]
                    )
                    st["osb"] = work.tile([P, D], F16, name="osb", bufs=2)
                ua.append(dma_a)
                for dt_ in range(D // QT):
                    for i in range(GRP):
                        def mm(dt_=dt_, i=i):
                            if i == 0:
                                st[f"op{dt_}"] = _pj_tile([P, QT], "op")
                            nc.tensor.matmul(
                                st[f"op{dt_}"][:],
                                st["attR"][:, i, :],
                                wo_sb[:, 2 * i, dt_ * QT : (dt_ + 1) * QT],
                                start=(i == 0),
                                stop=False,
                            )
                        ua.append(mm)
                def dma_b(qt=qt):
                    nc.sync.dma_start(
                        st["attR"][:, GRP : 2 * GRP, :],
                        cc_q[qt, :, GRP : 2 * GRP, bass.ds(qoff, P)],
                    )
                ub.append(dma_b)
                for dt_ in range(D // QT):
                    for i in range(GRP):
                        def mm(dt_=dt_, i=i):
                            nc.tensor.matmul(
                                st[f"op{dt_}"][:],
                                st["attR"][:, GRP + i, :],
                                wo_sb[:, 2 * i + 1, dt_ * QT : (dt_ + 1) * QT],
                                start=False,
                                stop=(i == GRP - 1),
                            )
                        ub.append(mm)
                    def cp(dt_=dt_):
                        nc.vector.tensor_copy(
                            st["osb"][:, dt_ * QT : (dt_ + 1) * QT], st[f"op{dt_}"][:]
                        )
                    ub.append(cp)
                def outdma(qt=qt):
                    nc.sync.dma_start(out_d[qt * P : (qt + 1) * P, :], st["osb"][:])
                ub.append(outdma)
                return ua, ub

            units = []

            def fill(rounds_left):
                if not units:
                    return
                n = max(1, (len(units) + rounds_left - 1) // max(rounds_left, 1))
                for _ in range(min(n, len(units))):
                    units.pop(0)()

            def flush():
                while units:
                    units.pop(0)()

            def norm_and_send(qt, pr, dens, accs, rb_tag=None):
                for h2 in range(2):
                    # broadcast f16 denominator row to 64 partitions (full-rate
                    # matmul), then fast-reciprocal at base partition 0 (the
                    # custom DVE op silently no-ops at nonzero base partitions)
                    if rb_tag is None:
                        rb_ps = _pj_tile([DH, QT], "rb")
                    else:
                        rb_ps = ps_pool.tile(
                            [DH, QT], f32, name="rb", tag=rb_tag, bufs=2
                        )
                    nc.tensor.matmul(
                        rb_ps[:],
                        ones_f16[DH : DH + 1, :],
                        dens[h2][DH : DH + 1, :],
                        start=True,
                        stop=True,
                    )
                    rb_sb = work.tile([DH, QT], f32, name="rb_sb", bufs=2)
                    nc.vector.reciprocal_approx_fast(rb_sb[:], rb_ps[:])
                    u_n = work.tile([DH, QT], F16, name="u_n", bufs=4)
                    # normalize straight from the PSUM accumulator (no uraw
                    # evacuation); the acc bank frees after this read
                    nc.vector.tensor_mul(u_n[:], accs[h2][0:DH, :], rb_sb[:])
                    row = qt * 2 * P + pr * P + h2 * DH
                    nc.sync.dma_start(cc_in[row : row + DH, :], u_n[:])

            def gather(qt, pr):
                ri = qt * 2 * P + pr * P
                ro = qt * GRP * 2 * P + pr * GRP * P
                nc.gpsimd.collective_compute(
                    "AllGather",
                    mybir.AluOpType.bypass,
                    replica_groups=[[0, 1, 2, 3], [4, 5, 6, 7]],
                    ins=[cc_in[ri : ri + P, :].opt()],
                    outs=[cc_out[ro : ro + GRP * P, :].opt()],
                )

            # ---------------- main loop ----------------
            op3b = [None]
            # upfront: only what qt0's pr0 pass needs; pr1's q/k chains become
            # the first fillers so the ACT starts ~3us earlier
            split0 = int(os.environ.get("KERNEL_SPLIT0", "1"))
            for u in proj_units(0, prs=(0,) if split0 else (0, 1)):
                u()
            for qt in range(NQT):
                if qt == 1:
                    nc.sync.dma_start(wo_sb[:], wof_d)
                # filler work for this quarter's ACT-bound attention span
                if qt == 0 and split0:
                    units.extend(proj_units(0, prs=(1,), with_v=False))
                if qt + 1 < NQT:
                    units.extend(proj_units(qt + 1))
                if qt == 2:
                    a0, b0 = out_proj_units(0)
                    units.extend(a0)
                    units.extend(b0)
                if qt == 3:
                    for q_ in (1, 2):
                        a_, b_ = out_proj_units(q_)
                        units.extend(a_)
                        units.extend(b_)

                q_sl = slice(qt * QT, (qt + 1) * QT)
                nk = (qt + 1) * (QT // P)
                rounds_left = 2 * nk
                pipe = int(os.environ.get("KERNEL_PIPE", "1"))
                tilepos = int(os.environ.get("KERNEL_TILEPOS", "1"))
                for pr in range(2):
                    acc = [
                        ps_pool.tile(
                            [DH + 1, QT], f32, name=f"acc{h2}", tag=f"acc{h2}", bufs=1
                        )
                        for h2 in range(2)
                    ]

                    def av_round(kb, r, pat):
                        for h2 in range(2):
                            nc.tensor.matmul(
                                acc[h2][0 : DH + 1, r:QT],
                                v_aug[pr * 2 + h2][:, kb, :],
                                pat[:, h2 * QT + r : (h2 + 1) * QT],
                                start=(kb == 0),
                                stop=(kb == nk - 1),
                            )

                    pend = None  # (kb, r, pat) awaiting its AV matmuls
                    for kb in range(nk):
                        k_sl = slice(kb * P, (kb + 1) * P)
                        ri = kb - qt * (QT // P)  # >= 0 on diagonal tiles
                        r = max(ri, 0) * P        # first valid col in this q tile
                        c_sl = slice(qt * QT + r, (qt + 1) * QT)
                        sc = ps_pool.tile([P, 2 * QT], f32, name="sc", tag="sc", bufs=2)
                        pat = work.tile([P, 2 * QT], F16, name="pat", bufs=3)
                        for h2 in range(2):
                            hb = h2 * DH
                            # explicit row-group placement: the two 64-row
                            # stationaries occupy disjoint halves of the PE
                            # array so their LDWEIGHTS+MATMULs can overlap
                            tp = (hb, 0) if tilepos else None
                            nc.tensor.matmul(
                                sc[:, h2 * QT + r : (h2 + 1) * QT],
                                kT[pr][hb : hb + DH, k_sl],
                                qT[pr][hb : hb + DH, c_sl],
                                start=True,
                                stop=True,
                                tile_position=tp,
                            )
                        # one exp covers both heads (cols 512..512+r of the
                        # diagonal rounds are stale-PSUM garbage, never read)
                        if exp_pair:
                            nc.scalar.activation(
                                pat[:, r : 2 * QT], sc[:, r : 2 * QT], Exp, scale=SCALE
                            )
                        else:
                            for h2 in range(2):
                                e_sl = slice(h2 * QT + r, (h2 + 1) * QT)
                                nc.scalar.activation(
                                    pat[:, e_sl], sc[:, e_sl], Exp, scale=SCALE
                                )
                        if ri >= 0:
                            for h2 in range(2):
                                nc.vector.tensor_mul(
                                    pat[:, h2 * QT + r : h2 * QT + r + P],
                                    pat[:, h2 * QT + r : h2 * QT + r + P],
                                    tri_sb[:],
                                )
                        # software-pipeline the PE stream one round deep: this
                        # round's score MMs enter the FIFO before the previous
                        # round's exp-gated AV MMs, so the PE never sits on the
                        # ACT semaphore with an exposed LDWEIGHTS behind it
                        if pipe:
                            if pend is not None:
                                av_round(*pend)
                            pend = (kb, r, pat)
                        else:
                            av_round(kb, r, pat)
                        rounds_left -= 1
                        fill(rounds_left)
                    if pend is not None:
                        av_round(*pend)
                    dens = []
                    for h2 in range(2):
                        # f16 denominator stays on lane 64 (DVE can't move
                        # across partitions); the matmul streams from there
                        den = work.tile([DH + 1, QT], F16, name="den", bufs=4)
                        nc.vector.tensor_copy(
                            den[DH : DH + 1, :], acc[h2][DH : DH + 1, :]
                        )
                        dens.append(den)
                    # the very last norm routes its broadcast through the idle
                    # score banks: the pj banks are held by out_proj(3) phase-a
                    # at that point and waiting on them would deadlock the
                    # release chain (rb -> gather(3,1) -> phase-b -> release)
                    last = qt == NQT - 1 and pr == 1
                    norm_and_send(qt, pr, dens, acc, rb_tag="sc" if last else None)
                    gather(qt, pr)
                    if qt == NQT - 1 and pr == 0:
                        a3, op3b[0] = out_proj_units(3)
                        units.extend(a3)
            flush()
            for u in op3b[0]:
                u()

    nc.compile()
    return nc


def _get_nc():
    if "nc" not in _CACHE:
        _CACHE["nc"] = _build()
    return _CACHE["nc"]


def _tri():
    k = np.arange(P)[:, None]
    q = np.arange(P)[None, :]
    return (q >= k).astype(np.float32)


def _ensure_ntff_hook():
    """Register the axon NTFF profile hook (missing antenv.axon_hooks shim)."""
    import types

    try:
        from antenv.axon_hooks import get_axon_ntff_profile_hook  # noqa: F401

        return
    except ImportError:
        pass
    import antenv

    if "/root/.axon_site" not in sys.path:
        sys.path.insert(0, "/root/.axon_site")
    from trn_agent_boot.trn_boot import _ntff_profile_via_ctypes

    hook = _ntff_profile_via_ctypes("/opt/axon/libaxon_pjrt.so")
    mod = types.ModuleType("antenv.axon_hooks")
    mod.get_axon_ntff_profile_hook = lambda: hook
    mod.set_axon_ntff_profile_hook = lambda h: None
    sys.modules["antenv.axon_hooks"] = mod
    antenv.axon_hooks = mod


def kernel(residual, W_Q, W_K, W_V, W_O):
    from concourse.bass_utils import run_bass_kernel_spmd

    if int(os.environ.get("KERNEL_TRACE", "0")):
        _ensure_ntff_hook()

    residual = np.ascontiguousarray(np.asarray(residual), np.float32)
    W_Q = np.ascontiguousarray(np.asarray(W_Q), np.float32)
    W_K = np.ascontiguousarray(np.asarray(W_K), np.float32)
    W_V = np.ascontiguousarray(np.asarray(W_V), np.float32)
    W_O = np.ascontiguousarray(np.asarray(W_O), np.float32)

    nc = _get_nc()
    tri = _tri()

    def chunked(a):
        # [D, n] -> [128, DC, n] so every DMA row is contiguous
        n = a.shape[1]
        return np.ascontiguousarray(
            a.reshape(DC, P, n).transpose(1, 0, 2).astype(np.float16)
        )

    wof = chunked(W_O.reshape(N_HEADS * DH, D))
    in_maps = []
    for c in range(NCORES):
        b, g = divmod(c, GRP)
        hs = slice(g * NH_CORE, (g + 1) * NH_CORE)
        in_maps.append(
            {
                "xt": chunked(residual[b].T),
                "wqt": chunked(W_Q[hs].transpose(2, 0, 1).reshape(D, NH_CORE * DH)),
                "wkt": chunked(W_K[hs].transpose(2, 0, 1).reshape(D, NH_CORE * DH)),
                "wvt": chunked(W_V[hs].transpose(2, 0, 1).reshape(D, NH_CORE * DH)),
                "wof": wof,
                "msk": tri.astype(np.float16),
            }
        )

    res = run_bass_kernel_spmd(
        nc,
        in_maps,
        core_ids=list(range(NCORES)),
        trace=bool(int(os.environ.get("KERNEL_TRACE", "0"))),
        trace_cores=(
            list(range(NCORES))
            if int(os.environ.get("KERNEL_TRACE_ALL", "0"))
            else [0] if int(os.environ.get("KERNEL_TRACE", "0")) else None
        ),
    )
    _CACHE["last_results"] = res

    out = np.empty((B, S, D), np.float32)
    for b in range(B):
        for g in range(GRP):
            blk = np.asarray(res.results[b * GRP + g]["out"], np.float32)
            for q in range(NQT):
                out[b, q * QT + g * P : q * QT + (g + 1) * P, :] = blk[
                    q * P : (q + 1) * P
                ]
    return out


# revision 49
# speedup vs baseline: 1.2086x; 1.2086x over previous
"""Trainium2 Bass kernel for causal multi-head attention (B=2, S=2048, D=1024, 16 heads x 64).

Sharding: 8 cores = 2 batches x 4 head-groups (tensor parallel over heads).
Each core computes attention for its 4 heads over the full sequence; the 4
cores of a batch AllGather normalized head outputs per q-quarter (4-rank
groups), and each core applies the full W_O projection to one 128-row block
of every quarter (so only the final AllGather gates a small slice of work).
Host concatenates the interleaved blocks.

Attention is flash-style with transposed scores: sT[k, q] = K Q^T (keys on
partitions). Both heads of a pair write one [128, 1024] 2-bank PSUM tile so a
single ACT exp covers them. AV uses stationary [v | 1] so PSUM row 64
accumulates the softmax denominator for free; the denominator row is
broadcast to 64 partitions by a 1-contraction fp32r matmul and reciprocated
at base partition 0 with the fast custom DVE op (nonzero-base custom DVE
silently no-ops). Head-pairs (pr) run sequentially per q-tile so attention
needs only 2 accumulator banks. QKV projection and out-projection matmul
chains are chopped into single-instruction units and interleaved between
attention rounds so the PE FIFO always has dependency-free filler during
ACT-bound stretches (keeps HAM warm, overlaps phases).
"""

import os
import sys

sys.path.insert(0, "/opt/trn_rl_repo")

import numpy as np

# ---- problem constants (hardcoded; kernel.py must be self-contained) ----
B = 2
S = 2048
D = 1024
N_HEADS = 16
DH = 64                 # head dim
NCORES = 8
NH_CORE = N_HEADS // 4  # 4 heads per core (4-way TP x 2-way batch DP)
SCALE = 1.0 / 8.0       # 1/sqrt(64)

P = 128                 # partitions
DC = D // P             # 8 contraction chunks for the projections
KC = S // P             # 16 key chunks
QT = 512                # q tile width (free dim) per quarter
NQT = S // QT           # 4 q tiles
GRP = 4                 # cores per batch group

_CACHE = {}


def _build():
    import concourse.bass as bass
    import concourse.tile as tile
    from concourse import bacc, mybir

    f32 = mybir.dt.float32
    f32r = mybir.dt.float32r
    F16 = mybir.dt.float16

    nc = bacc.Bacc(
        "TRN2",
        target_bir_lowering=False,
        debug=False,
        enable_asserts=False,
        num_devices=NCORES,
    )

    # all inputs pre-chunked host-side to [128, DC, n] so DMAs are contiguous
    xt_d = nc.dram_tensor("xt", [P, DC, S], F16, kind="ExternalInput").ap()
    wqt_d = nc.dram_tensor("wqt", [P, DC, NH_CORE * DH], F16, kind="ExternalInput").ap()
    wkt_d = nc.dram_tensor("wkt", [P, DC, NH_CORE * DH], F16, kind="ExternalInput").ap()
    wvt_d = nc.dram_tensor("wvt", [P, DC, NH_CORE * DH], F16, kind="ExternalInput").ap()
    wof_d = nc.dram_tensor("wof", [P, DC, D], F16, kind="ExternalInput").ap()
    msk_d = nc.dram_tensor("msk", [P, P], F16, kind="ExternalInput").ap()
    # 4 blocks of 128 rows: block q = rows [q*512 + g*128 .. +128] of this
    # core's batch output (g = group rank)
    out_d = nc.dram_tensor("out", [NQT * P, D], F16, kind="ExternalOutput").ap()
    exp_pair = int(os.environ.get("KERNEL_EXPPAIR", "1"))

    Exp = mybir.ActivationFunctionType.Exp

    with tile.TileContext(nc) as tc:
        with (
            tc.tile_pool(name="const", bufs=1) as const,
            tc.tile_pool(name="work", bufs=2) as work,
            tc.tile_pool(name="ps", bufs=1, space="PSUM") as ps_pool,
            tc.tile_pool(name="dram", bufs=1, space="DRAM") as dram,
        ):
            # ---------------- input DMAs ----------------
            wq_sb = const.tile([P, DC, NH_CORE * DH], F16)
            nc.sync.dma_start(wq_sb[:], wqt_d)
            wk_sb = const.tile([P, DC, NH_CORE * DH], F16)
            nc.sync.dma_start(wk_sb[:], wkt_d)

            # residual^T: first quarter per d-chunk (small, on gpsimd queue so
            # issue overlaps the sync-engine weight DMAs), rest in fat slices
            # issue the first-quarter chunks from two idle engine queues in
            # parallel (each dma_start costs ~0.6us of issue time on its
            # engine; serializing all 8 on one engine delays the first matmul)
            xt_sb = const.tile([P, DC, S], F16)
            for dc in range(DC):
                eng = nc.gpsimd if dc % 2 == 0 else nc.scalar
                eng.dma_start(xt_sb[:, dc, 0:QT], xt_d[:, dc, 0:QT])
            tri_sb = const.tile([P, P], F16)
            nc.sync.dma_start(tri_sb[:], msk_d)
            wv_sb = const.tile([P, DC, NH_CORE * DH], F16)
            nc.sync.dma_start(wv_sb[:], wvt_d)
            # pair chunks into wide transfers split across two queues, and
            # land quarter-1's columns first so the projection fillers that
            # run during quarter-0's attention aren't gated on DMA
            for cols in (slice(QT, 2 * QT), slice(2 * QT, S)):
                for dc in range(0, DC, 2):
                    eng = nc.gpsimd if dc % 4 == 0 else nc.scalar
                    eng.dma_start(
                        xt_sb[:, dc : dc + 2, cols], xt_d[:, dc : dc + 2, cols]
                    )

            # ---------------- SBUF state ----------------
            qT = [const.tile([P, S], F16, name=f"qT{i}") for i in range(2)]
            kT = [const.tile([P, S], F16, name=f"kT{i}") for i in range(2)]
            v_aug = [const.tile([P, KC, DH + 1], F16, name=f"vaug{h}") for h in range(NH_CORE)]
            ones_f32 = const.tile([P, DH], f32)
            nc.vector.memset(ones_f32[:], 1.0)
            ones_f16 = const.tile([DH + 1, DH], F16)
            nc.vector.memset(ones_f16[:], 1.0)
            for h in range(NH_CORE):
                nc.scalar.copy(v_aug[h][:, :, DH : DH + 1], ones_f32[:, 0:KC, None])

            wo_sb = const.tile([P, DC, D], F16)

            cc_in = dram.tile([NQT * 2 * P, QT], F16)
            cc_out = dram.tile([NQT * GRP * 2 * P, QT], F16)

            pj = [0]  # alternating tag counter for the 2 shared psum banks

            def _pj_tile(shape, name):
                t = ps_pool.tile(shape, f32, name=name, tag=f"pj{pj[0] % 2}", bufs=1)
                pj[0] += 1
                return t

            # ---- filler units: single instructions emitted between rounds ----
            def qk_chain_units(nt, pr, w_sb, dst):
                st = {}
                us = []
                for dc in range(DC):
                    def mm(dc=dc, nt=nt, pr=pr, w_sb=w_sb):
                        if dc == 0:
                            st["pp"] = _pj_tile([P, QT], "pp")
                        nc.tensor.matmul(
                            st["pp"][:],
                            w_sb[:, dc, pr * P : (pr + 1) * P],
                            xt_sb[:, dc, nt * QT : (nt + 1) * QT],
                            start=(dc == 0),
                            stop=(dc == DC - 1),
                        )
                    us.append(mm)
                def cp(nt=nt, dst=dst):
                    nc.vector.tensor_copy(dst[:, nt * QT : (nt + 1) * QT], st["pp"][:])
                us.append(cp)
                return us

            def v_chain_units(pc):
                st = {}
                us = []
                for dc in range(DC):
                    def mm(dc=dc, pc=pc):
                        if dc == 0:
                            st["vp"] = _pj_tile([P, NH_CORE * DH], "vp")
                        nc.tensor.matmul(
                            st["vp"][:],
                            xt_sb[:, dc, pc * P : (pc + 1) * P],
                            wv_sb[:, dc, :],
                            start=(dc == 0),
                            stop=(dc == DC - 1),
                        )
                    us.append(mm)
                for h in range(NH_CORE):
                    def cp(h=h, pc=pc):
                        nc.vector.tensor_copy(
                            v_aug[h][:, pc, 0:DH], st["vp"][:, h * DH : (h + 1) * DH]
                        )
                    us.append(cp)
                return us

            def proj_units(nt, prs=(0, 1), with_v=True):
                us = []
                for pr in prs:
                    us += qk_chain_units(nt, pr, wq_sb, qT[pr])
                    us += qk_chain_units(nt, pr, wk_sb, kT[pr])
                if with_v:
                    for pc in range(4 * nt, 4 * nt + 4):
                        us += v_chain_units(pc)
                return us

            # my 128-q-column block within each quarter (dynamic by rank)
            pid = nc.partition_id()
            qoff = nc.snap(
                nc.s_assert_within(
                    (pid % GRP) * P, 0, QT - P, skip_runtime_assert=True
                )
            )
            cc_q = cc_out[:].rearrange("(t a p) q -> t p a q", p=P, a=GRP * 2)

            def out_proj_units(qt):
                """Two phases: (a) after gather(qt, pr=0) — attR chunks 0-3
                (W_O chunks 0,2,4,6); (b) after gather(qt, pr=1) — chunks 4-7
                (W_O chunks 1,3,5,7) + evacuate + store. PSUM accumulates
                across the phase boundary."""
                st = {}
                ua, ub = [], []
                def dma_a(qt=qt):
                    st["attR"] = work.tile([P, GRP * 2, P], F16, name="attR", bufs=2)
                    nc.sync.dma_start(
                        st["attR"][:, 0:GRP, :], cc_q[qt, :, 0:GRP, bass.ds(qoff, P)]
                    )
                    st["osb"] = work.tile([P, D], F16, name="osb", bufs=2)
                ua.append(dma_a)
                for dt_ in range(D // QT):
                    for i in range(GRP):
                        def mm(dt_=dt_, i=i):
                            if i == 0:
                                st[f"op{dt_}"] = _pj_tile([P, QT], "op")
                            nc.tensor.matmul(
                                st[f"op{dt_}"][:],
                                st["attR"][:, i, :],
                                wo_sb[:, 2 * i, dt_ * QT : (dt_ + 1) * QT],
                                start=(i == 0),
                                stop=False,
                            )
                        ua.append(mm)
                def dma_b(qt=qt):
                    nc.sync.dma_start(
                        st["attR"][:, GRP : 2 * GRP, :],
                        cc_q[qt, :, GRP : 2 * GRP, bass.ds(qoff, P)],
                    )
                ub.append(dma_b)
                for dt_ in range(D // QT):
                    for i in range(GRP):
                        def mm(dt_=dt_, i=i):
                            nc.tensor.matmul(
                                st[f"op{dt_}"][:],
                                st["attR"][:, GRP + i, :],
                                wo_sb[:, 2 * i + 1, dt_ * QT : (dt_ + 1) * QT],
                                start=False,
                                stop=(i == GRP - 1),
                            )
                        ub.append(mm)
                    def cp(dt_=dt_):
                        nc.vector.tensor_copy(
                            st["osb"][:, dt_ * QT : (dt_ + 1) * QT], st[f"op{dt_}"][:]
                        )
                    ub.append(cp)
                def outdma(qt=qt):
                    nc.sync.dma_start(out_d[qt * P : (qt + 1) * P, :], st["osb"][:])
                ub.append(outdma)
                return ua, ub

            units = []

            def fill(rounds_left):
                if not units:
                    return
                n = max(1, (len(units) + rounds_left - 1) // max(rounds_left, 1))
                for _ in range(min(n, len(units))):
                    units.pop(0)()

            def flush():
                while units:
                    units.pop(0)()

            def norm_and_send(qt, pr, dens, accs, rb_tag=None):
                for h2 in range(2):
                    # broadcast f16 denominator row to 64 partitions (full-rate
                    # matmul), then fast-reciprocal at base partition 0 (the
                    # custom DVE op silently no-ops at nonzero base partitions)
                    if rb_tag is None:
                        rb_ps = _pj_tile([DH, QT], "rb")
                    else:
                        rb_ps = ps_pool.tile(
                            [DH, QT], f32, name="rb", tag=rb_tag, bufs=2
                        )
                    nc.tensor.matmul(
                        rb_ps[:],
                        ones_f16[DH : DH + 1, :],
                        dens[h2][DH : DH + 1, :],
                        start=True,
                        stop=True,
                    )
                    rb_sb = work.tile([DH, QT], f32, name="rb_sb", bufs=2)
                    nc.vector.reciprocal_approx_fast(rb_sb[:], rb_ps[:])
                    u_n = work.tile([DH, QT], F16, name="u_n", bufs=4)
                    # normalize straight from the PSUM accumulator (no uraw
                    # evacuation); the acc bank frees after this read
                    nc.vector.tensor_mul(u_n[:], accs[h2][0:DH, :], rb_sb[:])
                    row = qt * 2 * P + pr * P + h2 * DH
                    nc.sync.dma_start(cc_in[row : row + DH, :], u_n[:])

            def gather(qt, pr):
                ri = qt * 2 * P + pr * P
                ro = qt * GRP * 2 * P + pr * GRP * P
                nc.gpsimd.collective_compute(
                    "AllGather",
                    mybir.AluOpType.bypass,
                    replica_groups=[[0, 1, 2, 3], [4, 5, 6, 7]],
                    ins=[cc_in[ri : ri + P, :].opt()],
                    outs=[cc_out[ro : ro + GRP * P, :].opt()],
                )

            # ---------------- main loop ----------------
            op3b = [None]
            # upfront: only what qt0's pr0 pass needs; pr1's q/k chains become
            # the first fillers so the ACT starts ~3us earlier
            split0 = int(os.environ.get("KERNEL_SPLIT0", "1"))
            for u in proj_units(0, prs=(0,) if split0 else (0, 1)):
                u()
            for qt in range(NQT):
                if qt == 1:
                    nc.sync.dma_start(wo_sb[:], wof_d)
                # filler work for this quarter's ACT-bound attention span
                if qt == 0 and split0:
                    units.extend(proj_units(0, prs=(1,), with_v=False))
                if qt + 1 < NQT:
                    units.extend(proj_units(qt + 1))
                if qt == 2:
                    a0, b0 = out_proj_units(0)
                    units.extend(a0)
                    units.extend(b0)
                if qt == 3:
                    for q_ in (1, 2):
                        a_, b_ = out_proj_units(q_)
                        units.extend(a_)
                        units.extend(b_)

                q_sl = slice(qt * QT, (qt + 1) * QT)
                nk = (qt + 1) * (QT // P)
                rounds_left = 2 * nk
                pipe = int(os.environ.get("KERNEL_PIPE", "1"))
                tilepos = int(os.environ.get("KERNEL_TILEPOS", "1"))
                for pr in range(2):
                    acc = [
                        ps_pool.tile(
                            [DH + 1, QT], f32, name=f"acc{h2}", tag=f"acc{h2}", bufs=1
                        )
                        for h2 in range(2)
                    ]

                    def av_round(kb, r, pat):
                        for h2 in range(2):
                            nc.tensor.matmul(
                                acc[h2][0 : DH + 1, r:QT],
                                v_aug[pr * 2 + h2][:, kb, :],
                                pat[:, h2 * QT + r : (h2 + 1) * QT],
                                start=(kb == 0),
                                stop=(kb == nk - 1),
                            )

                    pend = None  # (kb, r, pat) awaiting its AV matmuls
                    for kb in range(nk):
                        k_sl = slice(kb * P, (kb + 1) * P)
                        ri = kb - qt * (QT // P)  # >= 0 on diagonal tiles
                        r = max(ri, 0) * P        # first valid col in this q tile
                        c_sl = slice(qt * QT + r, (qt + 1) * QT)
                        sc = ps_pool.tile([P, 2 * QT], f32, name="sc", tag="sc", bufs=2)
                        pat = work.tile([P, 2 * QT], F16, name="pat", bufs=3)
                        for h2 in range(2):
                            hb = h2 * DH
                            # explicit row-group placement: the two 64-row
                            # stationaries occupy disjoint halves of the PE
                            # array so their LDWEIGHTS+MATMULs can overlap
                            tp = (hb, 0) if tilepos else None
                            nc.tensor.matmul(
                                sc[:, h2 * QT + r : (h2 + 1) * QT],
                                kT[pr][hb : hb + DH, k_sl],
                                qT[pr][hb : hb + DH, c_sl],
                                start=True,
                                stop=True,
                                tile_position=tp,
                            )
                        # one exp covers both heads (cols 512..512+r of the
                        # diagonal rounds are stale-PSUM garbage, never read)
                        if exp_pair:
                            nc.scalar.activation(
                                pat[:, r : 2 * QT], sc[:, r : 2 * QT], Exp, scale=SCALE
                            )
                        else:
                            for h2 in range(2):
                                e_sl = slice(h2 * QT + r, (h2 + 1) * QT)
                                nc.scalar.activation(
                                    pat[:, e_sl], sc[:, e_sl], Exp, scale=SCALE
                                )
                        if ri >= 0:
                            for h2 in range(2):
                                nc.vector.tensor_mul(
                                    pat[:, h2 * QT + r : h2 * QT + r + P],
                                    pat[:, h2 * QT + r : h2 * QT + r + P],
                                    tri_sb[:],
                                )
                        # software-pipeline the PE stream one round deep: this
                        # round's score MMs enter the FIFO before the previous
                        # round's exp-gated AV MMs, so the PE never sits on the
                        # ACT semaphore with an exposed LDWEIGHTS behind it
                        if pipe:
                            if pend is not None:
                                av_round(*pend)
                            pend = (kb, r, pat)
                        else:
                            av_round(kb, r, pat)
                        rounds_left -= 1
                        fill(rounds_left)
                    if pend is not None:
                        av_round(*pend)
                    dens = []
                    for h2 in range(2):
                        # f16 denominator stays on lane 64 (DVE can't move
                        # across partitions); the matmul streams from there
                        den = work.tile([DH + 1, QT], F16, name="den", bufs=4)
                        nc.vector.tensor_copy(
                            den[DH : DH + 1, :], acc[h2][DH : DH + 1, :]
                        )
                        dens.append(den)
                    # the very last norm routes its broadcast through the idle
                    # score banks: the pj banks are held by out_proj(3) phase-a
                    # at that point and waiting on them would deadlock the
                    # release chain (rb -> gather(3,1) -> phase-b -> release)
                    last = qt == NQT - 1 and pr == 1
                    norm_and_send(qt, pr, dens, acc, rb_tag="sc" if last else None)
                    gather(qt, pr)
                    if qt == NQT - 1 and pr == 0:
                        a3, op3b[0] = out_proj_units(3)
                        units.extend(a3)
            flush()
            for u in op3b[0]:
                u()

    nc.compile()
    return nc


def _get_nc():
    if "nc" not in _CACHE:
        _CACHE["nc"] = _build()
    return _CACHE["nc"]


def _tri():
    k = np.arange(P)[:, None]
    q = np.arange(P)[None, :]
    return (q >= k).astype(np.float32)


def _ensure_ntff_hook():
    """Register the axon NTFF profile hook (missing antenv.axon_hooks shim)."""
    import types

    try:
        from antenv.axon_hooks import get_axon_ntff_profile_hook  # noqa: F401

        return
    except ImportError:
        pass
    import antenv

    if "/root/.axon_site" not in sys.path:
        sys.path.insert(0, "/root/.axon_site")
    from trn_agent_boot.trn_boot import _ntff_profile_via_ctypes

    hook = _ntff_profile_via_ctypes("/opt/axon/libaxon_pjrt.so")
    mod = types.ModuleType("antenv.axon_hooks")
    mod.get_axon_ntff_profile_hook = lambda: hook
    mod.set_axon_ntff_profile_hook = lambda h: None
    sys.modules["antenv.axon_hooks"] = mod
    antenv.axon_hooks = mod


def kernel(residual, W_Q, W_K, W_V, W_O):
    from concourse.bass_utils import run_bass_kernel_spmd

    if int(os.environ.get("KERNEL_TRACE", "0")):
        _ensure_ntff_hook()

    residual = np.ascontiguousarray(np.asarray(residual), np.float32)
    W_Q = np.ascontiguousarray(np.asarray(W_Q), np.float32)
    W_K = np.ascontiguousarray(np.asarray(W_K), np.float32)
    W_V = np.ascontiguousarray(np.asarray(W_V), np.float32)
    W_O = np.ascontiguousarray(np.asarray(W_O), np.float32)

    nc = _get_nc()
    tri = _tri()

    def chunked(a):
        # [D, n] -> [128, DC, n] so every DMA row is contiguous
        n = a.shape[1]
        return np.ascontiguousarray(
            a.reshape(DC, P, n).transpose(1, 0, 2).astype(np.float16)
        )

    wof = chunked(W_O.reshape(N_HEADS * DH, D))
    in_maps = []
    for c in range(NCORES):
        b, g = divmod(c, GRP)
        hs = slice(g * NH_CORE, (g + 1) * NH_CORE)
        in_maps.append(
            {
                "xt": chunked(residual[b].T),
                "wqt": chunked(W_Q[hs].transpose(2, 0, 1).reshape(D, NH_CORE * DH)),
                "wkt": chunked(W_K[hs].transpose(2, 0, 1).reshape(D, NH_CORE * DH)),
                "wvt": chunked(W_V[hs].transpose(2, 0, 1).reshape(D, NH_CORE * DH)),
                "wof": wof,
                "msk": tri.astype(np.float16),
            }
        )

    res = run_bass_kernel_spmd(
        nc,
        in_maps,
        core_ids=list(range(NCORES)),
        trace=bool(int(os.environ.get("KERNEL_TRACE", "0"))),
        trace_cores=(
            list(range(NCORES))
            if int(os.environ.get("KERNEL_TRACE_ALL", "0"))
            else [0] if int(os.environ.get("KERNEL_TRACE", "0")) else None
        ),
    )
    _CACHE["last_results"] = res

    out = np.empty((B, S, D), np.float32)
    for b in range(B):
        for g in range(GRP):
            blk = np.asarray(res.results[b * GRP + g]["out"], np.float32)
            for q in range(NQT):
                out[b, q * QT + g * P : q * QT + (g + 1) * P, :] = blk[
                    q * P : (q + 1) * P
                ]
    return out


# revision 50
# speedup vs baseline: 1.2775x; 1.0570x over previous
"""Trainium2 Bass kernel for causal multi-head attention (B=2, S=2048, D=1024, 16 heads x 64).

Sharding: 8 cores = 2 batches x 4 head-groups (tensor parallel over heads).
Each core computes attention for its 4 heads over the full sequence; the 4
cores of a batch AllGather normalized head outputs per q-quarter (4-rank
groups), and each core applies the full W_O projection to one 128-row block
of every quarter (so only the final AllGather gates a small slice of work).
Host concatenates the interleaved blocks.

Attention is flash-style with transposed scores: sT[k, q] = K Q^T (keys on
partitions). Both heads of a pair write one [128, 1024] 2-bank PSUM tile so a
single ACT exp covers them. AV uses stationary [v | 1] so PSUM row 64
accumulates the softmax denominator for free; the denominator row is
broadcast to 64 partitions by a 1-contraction fp32r matmul and reciprocated
at base partition 0 with the fast custom DVE op (nonzero-base custom DVE
silently no-ops). Head-pairs (pr) run sequentially per q-tile so attention
needs only 2 accumulator banks. QKV projection and out-projection matmul
chains are chopped into single-instruction units and interleaved between
attention rounds so the PE FIFO always has dependency-free filler during
ACT-bound stretches (keeps HAM warm, overlaps phases).
"""

import os
import sys

sys.path.insert(0, "/opt/trn_rl_repo")

import numpy as np

# ---- problem constants (hardcoded; kernel.py must be self-contained) ----
B = 2
S = 2048
D = 1024
N_HEADS = 16
DH = 64                 # head dim
NCORES = 8
NH_CORE = N_HEADS // 4  # 4 heads per core (4-way TP x 2-way batch DP)
SCALE = 1.0 / 8.0       # 1/sqrt(64)

P = 128                 # partitions
DC = D // P             # 8 contraction chunks for the projections
KC = S // P             # 16 key chunks
QT = 512                # q tile width (free dim) per quarter
NQT = S // QT           # 4 q tiles
GRP = 4                 # cores per batch group

_CACHE = {}


def _build():
    import concourse.bass as bass
    import concourse.tile as tile
    from concourse import bacc, mybir

    f32 = mybir.dt.float32
    f32r = mybir.dt.float32r
    F16 = mybir.dt.float16

    nc = bacc.Bacc(
        "TRN2",
        target_bir_lowering=False,
        debug=False,
        enable_asserts=False,
        num_devices=NCORES,
    )

    # all inputs pre-chunked host-side to [128, DC, n] so DMAs are contiguous
    xt_d = nc.dram_tensor("xt", [P, DC, S], F16, kind="ExternalInput").ap()
    wqt_d = nc.dram_tensor("wqt", [P, DC, NH_CORE * DH], F16, kind="ExternalInput").ap()
    wkt_d = nc.dram_tensor("wkt", [P, DC, NH_CORE * DH], F16, kind="ExternalInput").ap()
    wvt_d = nc.dram_tensor("wvt", [P, DC, NH_CORE * DH], F16, kind="ExternalInput").ap()
    wof_d = nc.dram_tensor("wof", [P, DC, D], F16, kind="ExternalInput").ap()
    msk_d = nc.dram_tensor("msk", [P, P], F16, kind="ExternalInput").ap()
    # 4 blocks of 128 rows: block q = rows [q*512 + g*128 .. +128] of this
    # core's batch output (g = group rank)
    out_d = nc.dram_tensor("out", [NQT * P, D], F16, kind="ExternalOutput").ap()
    exp_pair = int(os.environ.get("KERNEL_EXPPAIR", "1"))

    Exp = mybir.ActivationFunctionType.Exp

    with tile.TileContext(nc) as tc:
        with (
            tc.tile_pool(name="const", bufs=1) as const,
            tc.tile_pool(name="work", bufs=2) as work,
            tc.tile_pool(name="ps", bufs=1, space="PSUM") as ps_pool,
            tc.tile_pool(name="dram", bufs=1, space="DRAM") as dram,
        ):
            # ---------------- input DMAs ----------------
            wq_sb = const.tile([P, DC, NH_CORE * DH], F16)
            nc.sync.dma_start(wq_sb[:], wqt_d)
            wk_sb = const.tile([P, DC, NH_CORE * DH], F16)
            nc.sync.dma_start(wk_sb[:], wkt_d)

            # residual^T: first quarter per d-chunk (small, on gpsimd queue so
            # issue overlaps the sync-engine weight DMAs), rest in fat slices
            # issue the first-quarter chunks from two idle engine queues in
            # parallel (each dma_start costs ~0.6us of issue time on its
            # engine; serializing all 8 on one engine delays the first matmul)
            xt_sb = const.tile([P, DC, S], F16)
            for dc in range(DC):
                eng = nc.gpsimd if dc % 2 == 0 else nc.scalar
                eng.dma_start(xt_sb[:, dc, 0:QT], xt_d[:, dc, 0:QT])
            tri_sb = const.tile([P, P], F16)
            nc.sync.dma_start(tri_sb[:], msk_d)
            wv_sb = const.tile([P, DC, NH_CORE * DH], F16)
            nc.sync.dma_start(wv_sb[:], wvt_d)
            # pair chunks into wide transfers split across two queues, and
            # land quarter-1's columns first so the projection fillers that
            # run during quarter-0's attention aren't gated on DMA
            for cols in (slice(QT, 2 * QT), slice(2 * QT, S)):
                for dc in range(0, DC, 2):
                    eng = nc.gpsimd if dc % 4 == 0 else nc.scalar
                    eng.dma_start(
                        xt_sb[:, dc : dc + 2, cols], xt_d[:, dc : dc + 2, cols]
                    )

            # ---------------- SBUF state ----------------
            qT = [const.tile([P, S], F16, name=f"qT{i}") for i in range(2)]
            kT = [const.tile([P, S], F16, name=f"kT{i}") for i in range(2)]
            v_aug = [const.tile([P, KC, DH + 1], F16, name=f"vaug{h}") for h in range(NH_CORE)]
            ones_f32 = const.tile([P, DH], f32)
            nc.vector.memset(ones_f32[:], 1.0)
            ones_f16 = const.tile([DH + 1, DH], F16)
            nc.vector.memset(ones_f16[:], 1.0)
            for h in range(NH_CORE):
                nc.scalar.copy(v_aug[h][:, :, DH : DH + 1], ones_f32[:, 0:KC, None])

            wo_sb = const.tile([P, DC, D], F16)

            cc_in = dram.tile([NQT * 2 * P, QT], F16)
            cc_out = dram.tile([NQT * GRP * 2 * P, QT], F16)

            pj = [0]  # alternating tag counter for the 2 shared psum banks

            def _pj_tile(shape, name):
                t = ps_pool.tile(shape, f32, name=name, tag=f"pj{pj[0] % 2}", bufs=1)
                pj[0] += 1
                return t

            # ---- filler units: single instructions emitted between rounds ----
            def qk_chain_units(nt, pr, w_sb, dst):
                st = {}
                us = []
                for dc in range(DC):
                    def mm(dc=dc, nt=nt, pr=pr, w_sb=w_sb):
                        if dc == 0:
                            st["pp"] = _pj_tile([P, QT], "pp")
                        nc.tensor.matmul(
                            st["pp"][:],
                            w_sb[:, dc, pr * P : (pr + 1) * P],
                            xt_sb[:, dc, nt * QT : (nt + 1) * QT],
                            start=(dc == 0),
                            stop=(dc == DC - 1),
                        )
                    us.append(mm)
                def cp(nt=nt, dst=dst):
                    nc.vector.tensor_copy(dst[:, nt * QT : (nt + 1) * QT], st["pp"][:])
                us.append(cp)
                return us

            def v_chain_units(pc):
                st = {}
                us = []
                for dc in range(DC):
                    def mm(dc=dc, pc=pc):
                        if dc == 0:
                            st["vp"] = _pj_tile([P, NH_CORE * DH], "vp")
                        nc.tensor.matmul(
                            st["vp"][:],
                            xt_sb[:, dc, pc * P : (pc + 1) * P],
                            wv_sb[:, dc, :],
                            start=(dc == 0),
                            stop=(dc == DC - 1),
                        )
                    us.append(mm)
                for h in range(NH_CORE):
                    def cp(h=h, pc=pc):
                        nc.vector.tensor_copy(
                            v_aug[h][:, pc, 0:DH], st["vp"][:, h * DH : (h + 1) * DH]
                        )
                    us.append(cp)
                return us

            def proj_units(nt, prs=(0, 1), with_v=True):
                us = []
                for pr in prs:
                    us += qk_chain_units(nt, pr, wq_sb, qT[pr])
                    us += qk_chain_units(nt, pr, wk_sb, kT[pr])
                if with_v:
                    for pc in range(4 * nt, 4 * nt + 4):
                        us += v_chain_units(pc)
                return us

            # my 128-q-column block within each quarter (dynamic by rank)
            pid = nc.partition_id()
            qoff = nc.snap(
                nc.s_assert_within(
                    (pid % GRP) * P, 0, QT - P, skip_runtime_assert=True
                )
            )
            cc_q = cc_out[:].rearrange("(t a p) q -> t p a q", p=P, a=GRP * 2)

            def out_proj_units(qt):
                """Two phases: (a) after gather(qt, pr=0) — attR chunks 0-3
                (W_O chunks 0,2,4,6); (b) after gather(qt, pr=1) — chunks 4-7
                (W_O chunks 1,3,5,7) + evacuate + store. PSUM accumulates
                across the phase boundary."""
                st = {}
                ua, ub = [], []
                def dma_a(qt=qt):
                    st["attR"] = work.tile([P, GRP * 2, P], F16, name="attR", bufs=2)
                    nc.sync.dma_start(
                        st["attR"][:, 0:GRP, :], cc_q[qt, :, 0:GRP, bass.ds(qoff, P)]
                    )
                    st["osb"] = work.tile([P, D], F16, name="osb", bufs=2)
                ua.append(dma_a)
                for dt_ in range(D // QT):
                    for i in range(GRP):
                        def mm(dt_=dt_, i=i):
                            if i == 0:
                                st[f"op{dt_}"] = _pj_tile([P, QT], "op")
                            nc.tensor.matmul(
                                st[f"op{dt_}"][:],
                                st["attR"][:, i, :],
                                wo_sb[:, 2 * i, dt_ * QT : (dt_ + 1) * QT],
                                start=(i == 0),
                                stop=False,
                            )
                        ua.append(mm)
                def dma_b(qt=qt):
                    nc.sync.dma_start(
                        st["attR"][:, GRP : 2 * GRP, :],
                        cc_q[qt, :, GRP : 2 * GRP, bass.ds(qoff, P)],
                    )
                ub.append(dma_b)
                for dt_ in range(D // QT):
                    for i in range(GRP):
                        def mm(dt_=dt_, i=i):
                            nc.tensor.matmul(
                                st[f"op{dt_}"][:],
                                st["attR"][:, GRP + i, :],
                                wo_sb[:, 2 * i + 1, dt_ * QT : (dt_ + 1) * QT],
                                start=False,
                                stop=(i == GRP - 1),
                            )
                        ub.append(mm)
                    def cp(dt_=dt_):
                        nc.vector.tensor_copy(
                            st["osb"][:, dt_ * QT : (dt_ + 1) * QT], st[f"op{dt_}"][:]
                        )
                    ub.append(cp)
                def outdma(qt=qt):
                    nc.sync.dma_start(out_d[qt * P : (qt + 1) * P, :], st["osb"][:])
                ub.append(outdma)
                return ua, ub

            units = []

            def fill(rounds_left):
                if not units:
                    return
                n = max(1, (len(units) + rounds_left - 1) // max(rounds_left, 1))
                for _ in range(min(n, len(units))):
                    units.pop(0)()

            def flush():
                while units:
                    units.pop(0)()

            def norm_and_send(qt, pr, dens, accs, rb_tag=None):
                for h2 in range(2):
                    # broadcast f16 denominator row to 64 partitions (full-rate
                    # matmul), then fast-reciprocal at base partition 0 (the
                    # custom DVE op silently no-ops at nonzero base partitions)
                    if rb_tag is None:
                        rb_ps = _pj_tile([DH, QT], "rb")
                    else:
                        rb_ps = ps_pool.tile(
                            [DH, QT], f32, name="rb", tag=rb_tag, bufs=2
                        )
                    nc.tensor.matmul(
                        rb_ps[:],
                        ones_f16[DH : DH + 1, :],
                        dens[h2][DH : DH + 1, :],
                        start=True,
                        stop=True,
                    )
                    rb_sb = work.tile([DH, QT], f32, name="rb_sb", bufs=4)
                    nc.vector.reciprocal_approx_fast(rb_sb[:], rb_ps[:])
                    u_n = work.tile([DH, QT], F16, name="u_n", bufs=6)
                    # normalize straight from the PSUM accumulator (no uraw
                    # evacuation); the acc bank frees after this read
                    nc.vector.tensor_mul(u_n[:], accs[h2][0:DH, :], rb_sb[:])
                    row = qt * 2 * P + pr * P + h2 * DH
                    nc.sync.dma_start(cc_in[row : row + DH, :], u_n[:])

            def gather(qt, pr):
                ri = qt * 2 * P + pr * P
                ro = qt * GRP * 2 * P + pr * GRP * P
                nc.gpsimd.collective_compute(
                    "AllGather",
                    mybir.AluOpType.bypass,
                    replica_groups=[[0, 1, 2, 3], [4, 5, 6, 7]],
                    ins=[cc_in[ri : ri + P, :].opt()],
                    outs=[cc_out[ro : ro + GRP * P, :].opt()],
                )

            # ---------------- main loop ----------------
            op3b = [None]
            # upfront: only what qt0's pr0 pass needs; pr1's q/k chains become
            # the first fillers so the ACT starts ~3us earlier
            split0 = int(os.environ.get("KERNEL_SPLIT0", "1"))
            for u in proj_units(0, prs=(0,) if split0 else (0, 1)):
                u()
            for qt in range(NQT):
                if qt == 1:
                    nc.sync.dma_start(wo_sb[:], wof_d)
                # filler work for this quarter's ACT-bound attention span
                if qt == 0 and split0:
                    units.extend(proj_units(0, prs=(1,), with_v=False))
                if qt + 1 < NQT:
                    units.extend(proj_units(qt + 1))
                if qt == 2:
                    a0, b0 = out_proj_units(0)
                    units.extend(a0)
                    units.extend(b0)
                if qt == 3:
                    for q_ in (1, 2):
                        a_, b_ = out_proj_units(q_)
                        units.extend(a_)
                        units.extend(b_)

                q_sl = slice(qt * QT, (qt + 1) * QT)
                nk = (qt + 1) * (QT // P)
                rounds_left = 2 * nk
                pipe = int(os.environ.get("KERNEL_PIPE", "1"))
                tilepos = int(os.environ.get("KERNEL_TILEPOS", "1"))
                for pr in range(2):
                    acc = [
                        ps_pool.tile(
                            [DH + 1, QT], f32, name=f"acc{h2}", tag=f"acc{h2}", bufs=1
                        )
                        for h2 in range(2)
                    ]

                    def av_round(kb, r, pat):
                        for h2 in range(2):
                            nc.tensor.matmul(
                                acc[h2][0 : DH + 1, r:QT],
                                v_aug[pr * 2 + h2][:, kb, :],
                                pat[:, h2 * QT + r : (h2 + 1) * QT],
                                start=(kb == 0),
                                stop=(kb == nk - 1),
                            )

                    pend = None  # (kb, r, pat) awaiting its AV matmuls
                    for kb in range(nk):
                        k_sl = slice(kb * P, (kb + 1) * P)
                        ri = kb - qt * (QT // P)  # >= 0 on diagonal tiles
                        r = max(ri, 0) * P        # first valid col in this q tile
                        c_sl = slice(qt * QT + r, (qt + 1) * QT)
                        sc = ps_pool.tile([P, 2 * QT], f32, name="sc", tag="sc", bufs=2)
                        pat = work.tile([P, 2 * QT], F16, name="pat", bufs=4)
                        for h2 in range(2):
                            hb = h2 * DH
                            # explicit row-group placement: the two 64-row
                            # stationaries occupy disjoint halves of the PE
                            # array so their LDWEIGHTS+MATMULs can overlap
                            tp = (hb, 0) if tilepos else None
                            nc.tensor.matmul(
                                sc[:, h2 * QT + r : (h2 + 1) * QT],
                                kT[pr][hb : hb + DH, k_sl],
                                qT[pr][hb : hb + DH, c_sl],
                                start=True,
                                stop=True,
                                tile_position=tp,
                            )
                        # one exp covers both heads (cols 512..512+r of the
                        # diagonal rounds are stale-PSUM garbage, never read)
                        if exp_pair:
                            nc.scalar.activation(
                                pat[:, r : 2 * QT], sc[:, r : 2 * QT], Exp, scale=SCALE
                            )
                        else:
                            for h2 in range(2):
                                e_sl = slice(h2 * QT + r, (h2 + 1) * QT)
                                nc.scalar.activation(
                                    pat[:, e_sl], sc[:, e_sl], Exp, scale=SCALE
                                )
                        if ri >= 0:
                            for h2 in range(2):
                                nc.vector.tensor_mul(
                                    pat[:, h2 * QT + r : h2 * QT + r + P],
                                    pat[:, h2 * QT + r : h2 * QT + r + P],
                                    tri_sb[:],
                                )
                        # software-pipeline the PE stream one round deep: this
                        # round's score MMs enter the FIFO before the previous
                        # round's exp-gated AV MMs, so the PE never sits on the
                        # ACT semaphore with an exposed LDWEIGHTS behind it
                        if pipe:
                            if pend is not None:
                                av_round(*pend)
                            pend = (kb, r, pat)
                        else:
                            av_round(kb, r, pat)
                        rounds_left -= 1
                        fill(rounds_left)
                    if pend is not None:
                        av_round(*pend)
                    dens = []
                    for h2 in range(2):
                        # f16 denominator stays on lane 64 (DVE can't move
                        # across partitions); the matmul streams from there
                        den = work.tile([DH + 1, QT], F16, name="den", bufs=6)
                        nc.vector.tensor_copy(
                            den[DH : DH + 1, :], acc[h2][DH : DH + 1, :]
                        )
                        dens.append(den)
                    # the very last norm routes its broadcast through the idle
                    # score banks: the pj banks are held by out_proj(3) phase-a
                    # at that point and waiting on them would deadlock the
                    # release chain (rb -> gather(3,1) -> phase-b -> release)
                    last = qt == NQT - 1 and pr == 1
                    norm_and_send(qt, pr, dens, acc, rb_tag="sc" if last else None)
                    gather(qt, pr)
                    if qt == NQT - 1 and pr == 0:
                        a3, op3b[0] = out_proj_units(3)
                        units.extend(a3)
            flush()
            for u in op3b[0]:
                u()

    nc.compile()
    return nc


def _get_nc():
    if "nc" not in _CACHE:
        _CACHE["nc"] = _build()
    return _CACHE["nc"]


def _tri():
    k = np.arange(P)[:, None]
    q = np.arange(P)[None, :]
    return (q >= k).astype(np.float32)


def _ensure_ntff_hook():
    """Register the axon NTFF profile hook (missing antenv.axon_hooks shim)."""
    import types

    try:
        from antenv.axon_hooks import get_axon_ntff_profile_hook  # noqa: F401

        return
    except ImportError:
        pass
    import antenv

    if "/root/.axon_site" not in sys.path:
        sys.path.insert(0, "/root/.axon_site")
    from trn_agent_boot.trn_boot import _ntff_profile_via_ctypes

    hook = _ntff_profile_via_ctypes("/opt/axon/libaxon_pjrt.so")
    mod = types.ModuleType("antenv.axon_hooks")
    mod.get_axon_ntff_profile_hook = lambda: hook
    mod.set_axon_ntff_profile_hook = lambda h: None
    sys.modules["antenv.axon_hooks"] = mod
    antenv.axon_hooks = mod


def kernel(residual, W_Q, W_K, W_V, W_O):
    from concourse.bass_utils import run_bass_kernel_spmd

    if int(os.environ.get("KERNEL_TRACE", "0")):
        _ensure_ntff_hook()

    residual = np.ascontiguousarray(np.asarray(residual), np.float32)
    W_Q = np.ascontiguousarray(np.asarray(W_Q), np.float32)
    W_K = np.ascontiguousarray(np.asarray(W_K), np.float32)
    W_V = np.ascontiguousarray(np.asarray(W_V), np.float32)
    W_O = np.ascontiguousarray(np.asarray(W_O), np.float32)

    nc = _get_nc()
    tri = _tri()

    def chunked(a):
        # [D, n] -> [128, DC, n] so every DMA row is contiguous
        n = a.shape[1]
        return np.ascontiguousarray(
            a.reshape(DC, P, n).transpose(1, 0, 2).astype(np.float16)
        )

    wof = chunked(W_O.reshape(N_HEADS * DH, D))
    in_maps = []
    for c in range(NCORES):
        b, g = divmod(c, GRP)
        hs = slice(g * NH_CORE, (g + 1) * NH_CORE)
        in_maps.append(
            {
                "xt": chunked(residual[b].T),
                "wqt": chunked(W_Q[hs].transpose(2, 0, 1).reshape(D, NH_CORE * DH)),
                "wkt": chunked(W_K[hs].transpose(2, 0, 1).reshape(D, NH_CORE * DH)),
                "wvt": chunked(W_V[hs].transpose(2, 0, 1).reshape(D, NH_CORE * DH)),
                "wof": wof,
                "msk": tri.astype(np.float16),
            }
        )

    res = run_bass_kernel_spmd(
        nc,
        in_maps,
        core_ids=list(range(NCORES)),
        trace=bool(int(os.environ.get("KERNEL_TRACE", "0"))),
        trace_cores=(
            list(range(NCORES))
            if int(os.environ.get("KERNEL_TRACE_ALL", "0"))
            else [0] if int(os.environ.get("KERNEL_TRACE", "0")) else None
        ),
    )
    _CACHE["last_results"] = res

    out = np.empty((B, S, D), np.float32)
    for b in range(B):
        for g in range(GRP):
            blk = np.asarray(res.results[b * GRP + g]["out"], np.float32)
            for q in range(NQT):
                out[b, q * QT + g * P : q * QT + (g + 1) * P, :] = blk[
                    q * P : (q + 1) * P
                ]
    return out
